# revision 7
# baseline (speedup 1.0000x reference)
"""HAN layer (2-metapath GAT + semantic FC) on 8 Trainium2 NeuronCores.

Sharding: core c = (relation r = c//4, dst-quarter q = c%4). Each core owns
one relation's edges whose dst falls in its quarter of the node space, for
ALL 4 heads. Node features are replicated (every core projects the full
table); per-edge work is done with dma_gather (the fast SWDGE gather path)
over 768B bf16 table rows; aggregation is one matmul per 128-edge tile
against a baked dst-one-hot; softmax normalization + the small semantic FC
run on host (as in the original baseline).

Device program (identical across cores; per-core data differs):
  Phase A: table[n] = [f0|1|f1|1|f2|1|f3|1|el0..3|er0..3|pad] bf16 (768B)
           via h @ W_aug in bf16 (PSUM fp32), DMA'd to DRAM.
  er load: gather own-quarter dst rows at node-PAIR granularity (idx=dst//2
           fits int16), parity-select er columns -> er_all [128,NWQ,4].
  Phase B per dst window (128 dsts):
    stage dst-one-hot (doh) and its transpose (dohT), gather src rows in
    <=1024-index groups (two src halves so idx fits int16), then per group:
      er_edge = dohT^T @ er_win  (PE)   x = el + er_edge  (DVE)
      x = leakyrelu(x) (DVE)            rg = exp(x) -> bf16 (ACT)
      m = [feat|1] * rg  (one 4D-broadcast DVE op)
    per tile: wacc[128d, 260] += doh^T @ m  (PE, PSUM accumulate)
    drain wacc -> outp[window].
Output per core: outp [NWQ*128, 260] = per dst: 4 heads x [w*feat(64)|denom].
"""
import numpy as np
import ml_dtypes

BF = ml_dtypes.bfloat16

N = 50000
E = 800000
IN = 256
H = 4
D = 64
NEG = 0.2
P = 128
NWQ = 98            # merged dst windows per quarter
QSZ = NWQ * P       # 12544 dst slots per quarter
HALF = 25000        # src half split (int16 index limit)
ROW = 384           # bf16 elems per table row (768B)
FB = H * (D + 1)    # 260: 4 heads x [feat(64)|1]
GMAX = 8            # tiles per gather group (<=1024 indices)

_CACHE = {}
_LAST = {}


def _set_dims(n, nwq, half):
    """Test hook: shrink problem dims (n must be a multiple of 128*?)."""
    global N, NWQ, QSZ, HALF
    N, NWQ, HALF = n, nwq, half
    QSZ = NWQ * P


# ---------------------------------------------------------------- host prep
def _prep_core(src, dst, q):
    """Per (relation, quarter): window-split edge lists.
    Returns list over wi of dict(srcA, dlocA, srcB, dlocB)."""
    dlo = q * QSZ
    dhi = min(N, dlo + QSZ)
    sel = (dst >= dlo) & (dst < dhi)
    s = src[sel].astype(np.int64)
    d = (dst[sel] - dlo).astype(np.int64)
    order = np.argsort(d, kind="stable")
    s, d = s[order], d[order]
    ws = np.searchsorted(d, np.arange(NWQ) * P)
    we = np.searchsorted(d, np.arange(NWQ) * P + P)
    wins = []
    for wi in range(NWQ):
        sw = s[ws[wi]:we[wi]]
        dw = d[ws[wi]:we[wi]] - wi * P
        a = sw < HALF
        wins.append(dict(srcA=sw[a], dlocA=dw[a], srcB=sw[~a], dlocB=dw[~a]))
    return wins


def _merge_schedule(all_wins):
    """Shared schedule: per window, (tilesA, tilesB) = max over cores."""
    tA = np.zeros(NWQ, np.int64)
    tB = np.zeros(NWQ, np.int64)
    for wins in all_wins:
        for wi in range(NWQ):
            tA[wi] = max(tA[wi], -(-len(wins[wi]["srcA"]) // P))
            tB[wi] = max(tB[wi], -(-len(wins[wi]["srcB"]) // P))
    tA = np.maximum(tA, 1)              # every window has >=1 tile
    # global tile offsets: windows in order, halves A then B
    tile0 = np.zeros(NWQ, np.int64)
    np.cumsum((tA + tB)[:-1], out=tile0[1:])
    T = int((tA + tB).sum())
    # groups: per (wi, half): chunks of <=GMAX tiles
    groups = []                          # (wi, half, tile_start, k, first, last)
    for wi in range(NWQ):
        t = tile0[wi]
        total = tA[wi] + tB[wi]
        done = 0
        for half, th in ((0, tA[wi]), (1, tB[wi])):
            pos = 0
            while pos < th:
                k = int(min(GMAX, th - pos))
                groups.append((wi, half, int(t), k, done == 0,
                               done + k == total))
                t += k
                pos += k
                done += k
    maxtw = int((tA + tB).max())
    return dict(tA=tA, tB=tB, tile0=tile0, T=T, groups=groups, maxtw=maxtw)


def _bake_core(wins, sched, q):
    """Per-core baked arrays matching the merged schedule."""
    T = sched["T"]
    vsrc = np.zeros(T * P, np.int64)     # half-local row index (pad 0)
    vdl = np.full(T * P, -1, np.int64)   # dst local in window (pad -1)
    for wi in range(NWQ):
        t = sched["tile0"][wi]
        for half, tmax in ((0, sched["tA"][wi]), (1, sched["tB"][wi])):
            key = "A" if half == 0 else "B"
            s = wins[wi]["src" + key] - (0 if half == 0 else HALF)
            d = wins[wi]["dloc" + key]
            n = len(s)
            e0 = t * P
            vsrc[e0:e0 + n] = s
            vdl[e0:e0 + n] = d
            t += tmax
    doh = np.zeros((P, T * P), BF)
    dohT = np.zeros((P, T * P), BF)
    eidx = np.arange(T * P)
    val = vdl >= 0
    tile_of = eidx // P
    eloc = eidx % P
    doh[eloc[val], tile_of[val] * P + vdl[val]] = 1
    dohT[vdl[val], tile_of[val] * P + eloc[val]] = 1
    # idx: [128, 8T] int16 wrapped (i%16 -> partition, i//16 -> col), x8 cores
    v = vsrc.reshape(T, GMAX, 16)
    idx16 = v.transpose(2, 0, 1).reshape(16, T * GMAX).astype(np.int16)
    idx_d = np.tile(idx16, (8, 1))
    # er pair-gather indices for this quarter: idx = dst//2 (pairs), 98*128 ids
    dlo = q * QSZ
    dhi = min(N, dlo + QSZ)
    er_ids = np.zeros(QSZ, np.int64)
    er_ids[:dhi - dlo] = (np.arange(dlo, dhi)) // 2
    ev = er_ids.reshape(QSZ // P, GMAX, 16)
    er_idx = np.tile(ev.transpose(2, 0, 1).reshape(16, QSZ // 16).astype(np.int16),
                     (8, 1))
    return dict(idx=idx_d, doh=doh, dohT=dohT, er_idx=er_idx)


# ---------------------------------------------------------------- device
def _build_program(sched):
    import concourse.bacc as bacc
    import concourse.mybir as mybir
    from concourse.tile import TileContext
    dt = mybir.dt

    T = sched["T"]
    maxtw = sched["maxtw"]
    NWA = (N + P - 1) // P              # node windows for phase A
    ERG = QSZ // P // GMAX + (1 if QSZ // P % GMAX else 0)  # er gather groups

    nc = bacc.Bacc("TRN2", target_bir_lowering=False, debug=False,
                   num_devices=8, num_swdge_queues=4)
    h_bf = nc.declare_dram_parameter("h_bf", [IN, N], dt.bfloat16, isOutput=False)
    waug_in = nc.declare_dram_parameter("W_aug", [IN, IN + 2 * H], dt.bfloat16,
                                        isOutput=False)
    idx_in = nc.declare_dram_parameter("idx", [P, GMAX * T], dt.int16,
                                       isOutput=False)
    eridx_in = nc.declare_dram_parameter("er_idx", [P, QSZ // 16], dt.int16,
                                         isOutput=False)
    pmask_in = nc.declare_dram_parameter("pmask", [P, GMAX * H], dt.bfloat16,
                                         isOutput=False)
    doh_in = nc.declare_dram_parameter("doh", [P, T * P], dt.bfloat16,
                                       isOutput=False)
    dohT_in = nc.declare_dram_parameter("dohT", [P, T * P], dt.bfloat16,
                                        isOutput=False)
    outp = nc.declare_dram_parameter("outp", [QSZ, FB], dt.float32, isOutput=True)
    table = nc.dram_tensor("table", [N, ROW], dt.bfloat16)

    WAC = IN + 2 * H                    # 264 W_aug cols

    with TileContext(nc) as tc:
        with tc.tile_pool(name="const", bufs=1) as constp, \
             tc.tile_pool(name="ha", bufs=3) as hap, \
             tc.tile_pool(name="rowp", bufs=3) as rowp, \
             tc.tile_pool(name="psA", bufs=2, space="PSUM") as psA, \
             tc.tile_pool(name="erg", bufs=2) as ergp, \
             tc.tile_pool(name="dohp", bufs=2) as dohp, \
             tc.tile_pool(name="dohTp", bufs=2) as dohTp, \
             tc.tile_pool(name="gtp", bufs=6) as gtp, \
             tc.tile_pool(name="erps", bufs=2, space="PSUM") as erps, \
             tc.tile_pool(name="xp", bufs=3) as xp, \
             tc.tile_pool(name="mp", bufs=3) as mp, \
             tc.tile_pool(name="waccp", bufs=2, space="PSUM") as waccp, \
             tc.tile_pool(name="wsp", bufs=2) as wsp:

            # ---- constants ----
            waug = constp.tile([P, 2, WAC], dt.bfloat16, tag="waug")
            nc.sync.dma_start(out=waug[:],
                              in_=waug_in.ap().rearrange("(k p) f -> p k f", p=P))
            idx_s = constp.tile([P, GMAX * T], dt.int16, tag="idx")
            nc.sync.dma_start(out=idx_s[:], in_=idx_in.ap())
            eridx_s = constp.tile([P, QSZ // 16], dt.int16, tag="eridx")
            nc.sync.dma_start(out=eridx_s[:], in_=eridx_in.ap())
            er_all = constp.tile([P, NWQ, H], dt.bfloat16, tag="erall")
            nc.vector.memset(er_all[:], 0.0)
            pmask = constp.tile([P, GMAX, H], dt.bfloat16, tag="pmask")
            nc.sync.dma_start(out=pmask[:],
                              in_=pmask_in.ap().rearrange("p (k h) -> p k h", h=H))

            # ---- Phase A: build table ----
            for i in range(NWA):
                n0 = i * P
                nn = min(P, N - n0)
                ht = hap.tile([P, 2, P], dt.bfloat16, tag="ht")
                nc.sync.dma_start(
                    out=ht[:, :, :nn],
                    in_=h_bf.ap().rearrange("(k p) n -> p k n", p=P)[:, :, n0:n0 + nn])
                fps = psA.tile([P, WAC], dt.float32, space="PSUM", tag="fps")
                for k in range(2):
                    nc.tensor.matmul(out=fps[:nn, :], lhsT=ht[:, k, :nn],
                                     rhs=waug[:, k, :], start=(k == 0),
                                     stop=(k == 1))
                row = rowp.tile([P, ROW], dt.bfloat16, tag="row")
                nc.vector.memset(row[:], 1.0)
                nc.vector.tensor_copy(
                    out=row[:nn, 0:H * (D + 1)].rearrange("p (h g) -> p h g", h=H)[:, :, 0:D],
                    in_=fps[:nn, 0:IN].rearrange("p (h f) -> p h f", h=H))
                nc.vector.tensor_copy(out=row[:nn, 260:268], in_=fps[:nn, IN:IN + 8])
                nc.sync.dma_start(out=table[n0:n0 + nn, :], in_=row[:nn, :])

            # ---- er pair-gather: own-quarter dst er values ----
            pair_ap = table.ap().rearrange("(a b) c -> a (b c)", b=2)
            for g in range(ERG):
                k = min(GMAX, QSZ // P - g * GMAX)
                ni = k * P
                egt = ergp.tile([P, GMAX, 2 * ROW], dt.bfloat16, tag="egt")
                nc.gpsimd.dma_gather(
                    egt[:, :k, :], pair_ap,
                    eridx_s[:, g * GMAX * (P // 16):(g * GMAX + k) * (P // 16)],
                    ni, ni, 2 * ROW, queue_num=g % 4)
                # er_even at cols 264:268, er_odd at 648:652; select by parity
                w0 = g * GMAX
                sel = er_all[:, w0:w0 + k, :]
                nc.vector.tensor_tensor(
                    out=sel, in0=egt[:, :k, 648:652],
                    in1=egt[:, :k, 264:268], op=mybir.AluOpType.subtract)
                nc.vector.tensor_tensor(
                    out=sel, in0=sel, in1=pmask[:, :k, :],
                    op=mybir.AluOpType.mult)
                nc.vector.tensor_tensor(
                    out=sel, in0=sel, in1=egt[:, :k, 264:268],
                    op=mybir.AluOpType.add)

            # ---- Phase B ----
            for wi in range(NWQ):
                t0 = int(sched["tile0"][wi])
                tw = int(sched["tA"][wi] + sched["tB"][wi])
                doh_w = dohp.tile([P, maxtw, P], dt.bfloat16, tag="dohw")
                nc.sync.dma_start(out=doh_w[:, :tw, :],
                                  in_=doh_in.ap()[:, t0 * P:(t0 + tw) * P])
                dohT_w = dohTp.tile([P, maxtw, P], dt.bfloat16, tag="dohTw")
                nc.sync.dma_start(out=dohT_w[:, :tw, :],
                                  in_=dohT_in.ap()[:, t0 * P:(t0 + tw) * P])
                er_win = er_all[:, wi, :]
                wacc = waccp.tile([P, FB], dt.float32, space="PSUM", tag="wacc")

                for (gwi, half, gt0, k, first, last) in sched["groups"]:
                    if gwi != wi:
                        continue
                    ni = k * P
                    gt = gtp.tile([P, GMAX, ROW], dt.bfloat16, tag="gt")
                    base = table.ap()[0:HALF, :] if half == 0 \
                        else table.ap()[HALF:N, :]
                    nc.gpsimd.dma_gather(
                        gt[:, :k, :], base,
                        idx_s[:, gt0 * GMAX:(gt0 + k) * GMAX],
                        ni, ni, ROW, queue_num=gt0 % 4)
                    # er_edge per tile into PSUM [128, k*4]
                    er_ps = erps.tile([P, GMAX * H], dt.float32, space="PSUM",
                                      tag="erps")
                    for l in range(k):
                        jj = gt0 - t0 + l
                        nc.tensor.matmul(out=er_ps[:, l * H:(l + 1) * H],
                                         lhsT=dohT_w[:, jj, :], rhs=er_win,
                                         start=True, stop=True)
                    x = xp.tile([P, GMAX, H], dt.float32, tag="x")
                    nc.vector.tensor_tensor(
                        out=x[:, :k, :], in0=gt[:, :k, 260:264],
                        in1=er_ps[:, :k * H].rearrange("p (k h) -> p k h", h=H),
                        op=mybir.AluOpType.add)
                    nc.vector.scalar_tensor_tensor(
                        out=x[:, :k, :], in0=x[:, :k, :], scalar=NEG,
                        in1=x[:, :k, :], op0=mybir.AluOpType.mult,
                        op1=mybir.AluOpType.max)
                    rg = xp.tile([P, GMAX, H], dt.bfloat16, tag="rg")
                    nc.scalar.activation(out=rg[:, :k, :], in_=x[:, :k, :],
                                         func=mybir.ActivationFunctionType.Exp)
                    m = mp.tile([P, GMAX, FB], dt.bfloat16, tag="m")
                    nc.vector.tensor_tensor(
                        out=m[:, :k, :].rearrange("p k (h f) -> p k h f", h=H),
                        in0=gt[:, :k, 0:FB].rearrange("p k (h f) -> p k h f", h=H),
                        in1=rg[:, :k, :].to_broadcast([P, k, H, D + 1]),
                        op=mybir.AluOpType.mult)
                    for l in range(k):
                        jj = gt0 - t0 + l
                        nc.tensor.matmul(out=wacc[:], lhsT=doh_w[:, jj, :],
                                         rhs=m[:, l, :],
                                         start=(jj == 0), stop=(jj == tw - 1))
                ws = wsp.tile([P, FB], dt.float32, tag="ws")
                nc.vector.tensor_copy(out=ws[:], in_=wacc[:])
                nc.sync.dma_start(out=outp.ap()[wi * P:(wi + 1) * P, :], in_=ws[:])
    nc.compile()
    return nc


# ---------------------------------------------------------------- kernel
def kernel(h, Wg1, al1, ar1, b1, Wg2, al2, ar2, b2, Wfc, bfc,
           src1, dst1, src2, dst2):
    from concourse.bass_utils import run_bass_kernel_spmd

    h = np.asarray(h, np.float32)
    Ws = [np.asarray(Wg1, np.float32), np.asarray(Wg2, np.float32)]
    als = [np.asarray(al1, np.float32), np.asarray(al2, np.float32)]
    ars = [np.asarray(ar1, np.float32), np.asarray(ar2, np.float32)]
    bs = [np.asarray(b1, np.float32), np.asarray(b2, np.float32)]
    edges = [(np.asarray(src1), np.asarray(dst1)),
             (np.asarray(src2), np.asarray(dst2))]

    # per-core window splits
    all_wins = []
    for c in range(8):
        r, q = c // 4, c % 4
        all_wins.append(_prep_core(edges[r][0].astype(np.int64),
                                   edges[r][1].astype(np.int64), q))
    sched = _merge_schedule(all_wins)

    key = ("v2", sched["T"], tuple(sched["tA"]), tuple(sched["tB"]))
    if key not in _CACHE:
        _CACHE[key] = _build_program(sched)
    nc = _CACHE[key]

    h_bf = np.ascontiguousarray(h.T).astype(BF)
    in_maps = []
    for c in range(8):
        r, q = c // 4, c % 4
        bake = _bake_core(all_wins[c], sched, q)
        W = Ws[r]                                   # [256, 256] rows = h*64+f
        W_aug = np.zeros((IN, IN + 2 * H), np.float32)
        W_aug[:, :IN] = W.T
        for hd in range(H):
            W_h = W[hd * D:(hd + 1) * D, :]         # [64, 256]
            W_aug[:, IN + hd] = W_h.T @ als[r][hd]
            W_aug[:, IN + H + hd] = W_h.T @ ars[r][hd]
        pmask = np.broadcast_to((np.arange(P) % 2).astype(BF)[:, None],
                                (P, GMAX * H)).copy()
        in_maps.append({
            "h_bf": h_bf,
            "W_aug": W_aug.astype(BF),
            "idx": bake["idx"],
            "er_idx": bake["er_idx"],
            "doh": bake["doh"],
            "dohT": bake["dohT"],
            "pmask": pmask,
        })

    _LAST["nc"] = nc
    _LAST["in_maps"] = in_maps
    res = run_bass_kernel_spmd(nc, in_maps, list(range(8)))

    os = []
    for r in range(2):
        o = np.zeros((N, IN), np.float32)
        for q in range(4):
            raw = res.results[r * 4 + q]["outp"]    # [QSZ, 260]
            dlo = q * QSZ
            nq = min(N, dlo + QSZ) - dlo
            blk = raw[:nq].reshape(nq, H, D + 1)
            o[dlo:dlo + nq] = (blk[:, :, :D] /
                               (blk[:, :, D:D + 1] + 1e-30)).reshape(nq, IN)
        os.append(o + bs[r][None, :])
    sem = np.concatenate(os, axis=1)                # [N, 512]
    Wfc = np.asarray(Wfc, np.float32)
    out = sem @ Wfc.T + np.asarray(bfc, np.float32)
    return out.astype(np.float32)
